# revision 29
# baseline (speedup 1.0000x reference)
"""Trainium2 Bass kernel for nn_MCQuantiles (ThreeCompNode SNN scan).

Strategy (8 NeuronCores, data-parallel over batch):
- Each core takes 8 batches x 32 samples = 256 rows of the B*S axis.
- Everything runs in "transposed space": feature dims on SBUF partitions,
  batch-rows on the free dim. All transposes/swizzles are done host-side for
  free; every DMA is a flat contiguous [128, X] block.
- The apical matmul (81% of FLOPs) runs in fp8e4 DoubleRow mode (K=256 per
  instruction, 2 MACs/cell/cycle). Wa is pre-scaled by 64 host-side to avoid
  e4m3 subnormals; the 1/64 descale folds into the DVE update constants.
  Margin analysis (sim_fp8.py): layer-2 membrane max ~0.35 vs threshold 0.5,
  so fp8 noise (~0.02) cannot flip any output spike.
- Membrane recurrences use 2^t-scaled state so each update is a single fused
  op; states are fused [128, 4R] tiles (4 feature groups side by side) so one
  DVE/GpSimd instruction covers all groups.
- Engine split: DVE does psum-sourced updates + thresholds, GpSimd does the
  SBUF-only adds/resets, ACT adds the per-partition c1 bias.
- W1/W2/basal matmuls stay bf16 (fp8 would make them LDWEIGHTS-bound).
- out accumulates in a persistent PSUM bank over all T, evicted once with
  scale 1/T + bias b2.
"""
import numpy as np
import ml_dtypes

import bass_rust
import concourse.bass as bass
import concourse.mybir as mybir
from concourse.bass_utils import run_bass_kernel_spmd
from concourse.tile import TileContext
from concourse.tile_rust import add_dep_helper

# ----- problem constants (hardcoded per contract) -----
T, B, S = 8, 64, 32
DS = DT = 3136
F = H = 512
L = 18
N_CORES = 8
NB = B // N_CORES              # 8 batches per core
R = NB * S                     # 256 rows per core
NPAIR = T // 2                 # 4 step pairs
NG = F // 128                  # 4 f-tiles (= h-tiles)
WSCALE = 64.0                  # host-side fp8 weight scale (2^6, exact)

# apical path: fp8 DoubleRow, k-tiles of 256 (3136 -> 3328 = 13*256)
NK2 = 13
KD2 = NK2 * 256
# basal path: bf16, k-tiles of 128 (3136 -> 3200 = 25*128)
NKB = 25
KDB = NKB * 128

# column offsets inside wallM [128, *] (bf16)
O_WB = 0                       # basal weights, NKB*F cols
O_SE = O_WB + NKB * F          # state embeddings, NKB*T*NB cols
WM_COLS = O_SE + NKB * T * NB
# wallB (bf16): W1.T, W2.T, then the -c1 aug row (K=1 fifth W1 k-tile)
O_W1 = 0
O_W2 = O_W1 + NG * H
O_AUG = O_W2 + NG * L
WB_COLS = O_AUG + NG * 128

F32 = mybir.dt.float32
BF16 = mybir.dt.bfloat16
F8 = mybir.dt.float8e4
OP = mybir.AluOpType
DR = mybir.MatmulPerfMode.DoubleRow

# apical DMA chunks, in k256 units (total NK2=13); chunk 0 small so the
# first matmul can start early; chunks 2/3 sized ~3.2us of PE work so the
# interleave filler covers the ~3.3us membrane-chain latency
CHUNKS2 = [1, 2, 3, 3, 2, 2]
CH2_OFF = [0, 1, 3, 6, 9, 11]
NCHUNK = len(CHUNKS2)


def _patch_tile_drain():
    """This walrus build allows a single sync-wait per TPB_CTRL Drain; Tile's
    kernel-tail drain attaches one wait per active logical proc. Split them
    across a chain of drains."""
    def _patched(self, tick_clock, wait_clock):
        nc = self.nc
        drain_inst = nc.sync.drain()
        wait_clock.add_sem_waits(
            drain_inst.ins, bass_rust.ScopedClock({None: tick_clock.global_clock})
        )
        si = drain_inst.ins.sync_info
        if si is not None and len(si.on_wait) > 1:
            waits = list(si.on_wait)
            drain_inst.ins.sync_info = mybir.SyncInfo(
                on_wait=waits[:1], on_update=list(si.on_update)
            )
            for w in waits[1:]:
                extra = nc.sync.drain()
                extra.ins.sync_info = mybir.SyncInfo(on_wait=[w], on_update=[])
        nc.all_engine_barrier()
        popped = nc._tile_sem_poison_stack.pop()
        assert popped is self._sem_poison
        nc.clear_and_free_semaphores(list(self.sems.allocated().values()))
        nc.all_engine_barrier()

    TileContext._drain_and_barrier = _patched


def _split_excess_waits(nc, limit=1):
    """Walrus here rejects instructions carrying more than ~1 sync-wait. Move
    excess waits onto same-engine NoOps inserted just before the instruction."""
    for fn in nc.m.functions:
        for bb in fn.blocks:
            new = []
            changed = False
            for inst in bb.instructions:
                si = getattr(inst, "sync_info", None)
                ow = list(si.on_wait) if si is not None and si.on_wait else []
                if len(ow) > limit:
                    extra = ow[limit:]
                    for j in range(0, len(extra), limit):
                        nop = mybir.InstNoOp(
                            name=f"{inst.name}-ws{j}", ins=[], outs=[]
                        )
                        nop.engine = inst.engine
                        nop.sync_info = mybir.SyncInfo(
                            on_wait=extra[j : j + limit], on_update=[]
                        )
                        new.append(nop)
                    inst.sync_info = mybir.SyncInfo(
                        on_wait=ow[:limit], on_update=list(si.on_update)
                    )
                    changed = True
                new.append(inst)
            if changed:
                try:
                    bb.instructions[:] = new
                except TypeError:
                    bb.instructions = new


def build_nc():
    _patch_tile_drain()
    nc = bass.Bass()

    teT = nc.declare_dram_parameter("teT", [NPAIR, 128, NK2 * 1024], F8, isOutput=False)
    wallA = nc.declare_dram_parameter("wallA", [128, NK2 * 1024], F8, isOutput=False)
    wallM = nc.declare_dram_parameter("wallM", [128, WM_COLS], BF16, isOutput=False)
    wallB = nc.declare_dram_parameter("wallB", [128, WB_COLS], BF16, isOutput=False)
    # cons: [0] = c2+b2 eviction bias (rows < L); [1] = 0.0; [2+t] = 2^t
    cons = nc.declare_dram_parameter("cons", [128, 2 + T], F32, isOutput=False)
    out = nc.declare_dram_parameter("out", [L, R], F32, isOutput=True)

    with TileContext(nc) as tc:
        with (
            tc.tile_pool(name="wpool", bufs=1) as wpool,
            tc.tile_pool(name="tepool", bufs=2) as tepool,
            tc.tile_pool(name="state", bufs=1) as state,
            tc.tile_pool(name="qpool", bufs=2) as qpool,
            tc.tile_pool(name="appool", bufs=1, space="PSUM") as appool,
            tc.tile_pool(name="hqpool", bufs=1, space="PSUM") as hqpool,
            tc.tile_pool(name="bopool", bufs=1, space="PSUM") as bopool,
        ):
            # ---- resident weights/constants ----
            wallA_c = []
            prev = None
            for c in range(NCHUNK):
                ck = wpool.tile(
                    [128, CHUNKS2[c] * 1024], F8, tag=f"wallA{c}", name=f"wa_ck{c}"
                )
                wallA_c.append(ck)
                d = nc.sync.dma_start(
                    ck[:],
                    wallA[:, CH2_OFF[c] * 1024 : (CH2_OFF[c] + CHUNKS2[c]) * 1024],
                )
                if prev is not None:
                    add_dep_helper(d.ins, prev.ins, reason="serialize wallA chunks")
                prev = d
            wallM_sb = wpool.tile([128, WM_COLS], BF16, tag="wallM", name="wallM_sb")
            wallB_sb = wpool.tile([128, WB_COLS], BF16, tag="wallB", name="wallB_sb")
            cons_sb = wpool.tile([128, 2 + T], F32, tag="cons", name="cons_sb")

            def waT8(c, kk, g):
                """fp8 DoubleRow lhsT [128, 2, 128] for local k256 kk, group g."""
                base = kk * 1024 + g * 256
                return wallA_c[c][:, base : base + 256].rearrange(
                    "p (two f) -> p two f", two=2
                )

            def wbT(k, g):
                return wallM_sb[:, O_WB + k * F + g * 128 : O_WB + k * F + (g + 1) * 128]

            def seT(k):
                return wallM_sb[:, O_SE + k * T * NB : O_SE + (k + 1) * T * NB]

            def w1T(k, g):
                return wallB_sb[:, O_W1 + k * H + g * 128 : O_W1 + k * H + (g + 1) * 128]

            def w2T(k):
                return wallB_sb[:, O_W2 + k * L : O_W2 + (k + 1) * L]

            def waug(g):  # [1, 128] fifth-k-tile row: -c1 slice
                return wallB_sb[0:1, O_AUG + g * 128 : O_AUG + (g + 1) * 128]

            ev_bias = cons_sb[0:L, 0:1]          # b2
            zero_ap = cons_sb[:, 1:2]

            def th2_ap(t):                       # 2^t column
                return cons_sb[:, 2 + t : 3 + t]

            # ---- fused state tiles ([128, 4R]: 4 groups side by side) ----
            A = [state.tile([128, NG * R], BF16, tag=f"A{p}", name=f"A{p}")
                 for p in range(2)]          # alpha ping-pong (even/odd t)
            M = state.tile([128, NG * R], BF16, tag="M", name="M")
            ML = state.tile([128, NG * R], BF16, tag="ML", name="ML")
            Bsc = state.tile([128, T * NG * NB], BF16, tag="Bsc", name="Bsc")
            ones = state.tile([1, R], BF16, tag="ones", name="ones")
            nc.vector.memset(ones[:], 1.0)

            o_psum = bopool.tile([L, R], F32, tag="o", name="o_psum")

            def A3(p):
                return A[p].rearrange("p (g r) -> p g r", g=NG)

            def B_bc(t, g):
                base = t * NG * NB + g * NB
                return (
                    Bsc[:, base : base + NB]
                    .unsqueeze(2)
                    .broadcast_to([128, NB, S])
                )

            def M_bs(g):
                return M[:, g * R : (g + 1) * R].rearrange(
                    "p (b s) -> p b s", s=S
                )

            # ---- emission helpers ----
            def emit_te_dma(pair, chain):
                tiles = []
                prev = None
                for c in range(NCHUNK):
                    tck = tepool.tile(
                        [128, CHUNKS2[c] * 1024], F8, tag=f"te{c}", name=f"te_ck{c}"
                    )
                    tiles.append(tck)
                    d = nc.sync.dma_start(
                        tck[:],
                        teT[pair][:, CH2_OFF[c] * 1024
                                  : (CH2_OFF[c] + CHUNKS2[c]) * 1024],
                    )
                    if prev is not None and chain:
                        add_dep_helper(d.ins, prev.ins,
                                       reason="serialize te chunk DMAs")
                    prev = d
                return tiles, prev

            def emit_ap_chunk(mega, te_tiles, c):
                for g in range(NG):
                    for kk in range(CHUNKS2[c]):
                        k = CH2_OFF[c] + kk
                        rhs = te_tiles[c][:, kk * 1024 : (kk + 1) * 1024].rearrange(
                            "p (two n) -> p two n", two=2
                        )
                        nc.tensor.matmul(
                            mega[:, g * 512 : (g + 1) * 512],
                            lhsT=waT8(c, kk, g),
                            rhs=rhs,
                            start=(k == 0),
                            stop=(k == NK2 - 1),
                            perf_mode=DR,
                        )

            def emit_a_updates(mega, pair):
                """alpha_t = alpha_{t-1} + (2^{t-1}/64) * ap64_t, fused over
                groups; ping-pong A[t%2]."""
                mega3 = mega.rearrange("p (g x) -> p g x", g=NG)
                for sub in range(2):
                    t = 2 * pair + sub
                    s_t = float(2 ** (t - 1)) / WSCALE
                    ap3 = mega3[:, :, sub * R : (sub + 1) * R]
                    if t == 0:
                        nc.vector.tensor_scalar(A3(0), ap3, s_t, None, OP.mult)
                    else:
                        nc.vector.scalar_tensor_tensor(
                            A3(t % 2), ap3, s_t, A3(1 - t % 2), OP.mult, OP.add
                        )

            def emit_basal():
                bs_ps = bopool.tile([128, NG * T * NB], F32, tag="bs", name="bs_ps")
                for g in range(NG):
                    for k in range(NKB):
                        nc.tensor.matmul(
                            bs_ps[:, g * T * NB : (g + 1) * T * NB],
                            lhsT=wbT(k, g),
                            rhs=seT(k),
                            start=(k == 0),
                            stop=(k == NKB - 1),
                        )
                ps3 = bs_ps.rearrange("p (g tb) -> p g tb", g=NG)
                for t in range(T):
                    src = ps3[:, :, t * NB : (t + 1) * NB]
                    dst = Bsc[:, t * NG * NB : (t + 1) * NG * NB].rearrange(
                        "p (g b) -> p g b", g=NG
                    )
                    if t == 0:
                        nc.vector.tensor_scalar(dst, src, 0.5, None, OP.mult)
                    else:
                        prv = Bsc[:, (t - 1) * NG * NB : t * NG * NB].rearrange(
                            "p (g b) -> p g b", g=NG
                        )
                        nc.vector.scalar_tensor_tensor(
                            dst, src, float(2 ** (t - 1)), prv, OP.mult, OP.add
                        )

            def emit_sub_m(pair, sub):
                """Membrane phase: DVE-only (M update, q, reset)."""
                t = 2 * pair + sub
                th_t = float(2 ** (t + 1))
                Ax = A[t % 2]
                # --- membrane: M = M_post + A_t + B_t (per-group DVE ops) ---
                for g in range(NG):
                    sl = slice(g * R, (g + 1) * R)
                    if t == 0:
                        nc.vector.tensor_tensor(
                            M_bs(g), A[0][:, sl].rearrange("p (b s) -> p b s", s=S),
                            B_bc(0, g), OP.add,
                        )
                    else:
                        nc.vector.tensor_tensor(
                            M[:, sl], M[:, sl], Ax[:, sl], OP.add
                        )
                        nc.vector.tensor_tensor(
                            M_bs(g), B_bc(t, g), M_bs(g), OP.add
                        )
                # spikes (DVE): q = NOT(spike) = (M <= th)
                qg = qpool.tile([128, NG * R], BF16, tag="q", name="qg")
                nc.vector.tensor_scalar(qg[:], M[:], th_t, None, OP.is_le)
                if t < T - 1:
                    # hard reset (DVE): M *= q  (dead at the last step)
                    nc.vector.tensor_tensor(M[:], M[:], qg[:], OP.mult)
                return qg

            def emit_sub_h(pair, sub, qg):
                """Matmul + LIF phase (PE/ACT/DVE)."""
                t = 2 * pair + sub
                sc_t = float(2 ** t)
                # --- layer 1: hq = q @ W1.T - c1 (bf16, fused psum); the -c1
                # aug matmul seeds each group (start=True) so it runs while
                # the PE would otherwise wait for q ---
                hq = hqpool.tile([128, NG * R], F32, tag="hq", name="hq")
                # hq spans 2 psum banks (g0,g1 | g2,g3): only one pending
                # accumulation group per bank, so seed bank-disjoint pairs.
                for gs in ((0, 2), (1, 3)):
                    for g in gs:
                        nc.tensor.matmul(
                            hq[:, g * R : (g + 1) * R],
                            lhsT=waug(g), rhs=ones[:],
                            start=True, stop=False,
                        )
                    for g in gs:
                        for k in range(NG):
                            nc.tensor.matmul(
                                hq[:, g * R : (g + 1) * R],
                                lhsT=w1T(k, g),
                                rhs=qg[:, k * R : (k + 1) * R],
                                start=False,
                                stop=(k == NG - 1),
                            )

                # --- LIF: ML = ML - 2^t * hq' ---
                if t == 0:
                    nc.vector.tensor_scalar(ML[:], hq[:], -1.0, None, OP.mult)
                else:
                    nc.vector.scalar_tensor_tensor(
                        ML[:], hq[:], -sc_t, ML[:], OP.mult, OP.add
                    )
                # qs = Sign(2^t - ML) in {-1,0,1}; qL = Relu(qs) = NOT(spike2);
                # sp2 = Relu(-qs). sp2 (not a qL-fold) keeps out EXACTLY zero
                # when no spike fires.
                qs = qpool.tile([128, NG * R], BF16, tag="qs", name="qs")
                nc.scalar.activation(
                    qs[:], ML[:], mybir.ActivationFunctionType.Sign,
                    bias=th2_ap(t), scale=-1.0,
                )
                spg = qpool.tile([128, NG * R], BF16, tag="sp2", name="spg")
                nc.scalar.activation(
                    spg[:], qs[:], mybir.ActivationFunctionType.Relu,
                    bias=zero_ap, scale=-1.0,
                )
                if t < T - 1:
                    # qL + LIF reset are dead at the last step
                    qL = qpool.tile([128, NG * R], BF16, tag="qL", name="qL")
                    nc.scalar.activation(
                        qL[:], qs[:], mybir.ActivationFunctionType.Relu,
                        bias=zero_ap, scale=1.0,
                    )
                    nc.vector.tensor_tensor(ML[:], ML[:], qL[:], OP.mult)

                # --- layer 2: out += sp2 @ W2.T (persistent psum) ---
                for k in range(NG):
                    nc.tensor.matmul(
                        o_psum[:],
                        lhsT=w2T(k),
                        rhs=spg[:, k * R : (k + 1) * R],
                        start=(t == 0 and k == 0),
                        stop=(t == T - 1 and k == NG - 1),
                    )

            # ---- prologue: pair 0 load + apical ----
            te_tiles, last_te_dma = emit_te_dma(0, chain=True)
            # HAM warmup: ~4us of dummy matmuls on the ones tile during the
            # otherwise-idle input-DMA window, so real matmuls start at 2.4GHz
            warm = hqpool.tile([128, NG * R], F32, tag="hq", name="warmup")
            for _ in range(12):
                nc.tensor.matmul(
                    warm[:, 0:R], lhsT=ones[0:1, 0:128], rhs=ones[:],
                    start=True, stop=True,
                )
            mega = appool.tile([128, NG * 512], F32, tag="ap", name="ap_mega")
            for c in range(NCHUNK):
                emit_ap_chunk(mega, te_tiles, c)
            dM = nc.sync.dma_start(wallM_sb[:], wallM[:])
            add_dep_helper(dM.ins, last_te_dma.ins, reason="wallM after te0 chain")
            dB = nc.sync.dma_start(wallB_sb[:], wallB[:])
            add_dep_helper(dB.ins, dM.ins, reason="wallB after wallM")
            dC = nc.sync.dma_start(cons_sb[:], cons[:])
            add_dep_helper(dC.ins, dM.ins, reason="cons after wallM")
            emit_basal()

            # ---- software-pipelined main loop ----
            for pair in range(NPAIR):
                emit_a_updates(mega, pair)
                if pair + 1 < NPAIR:
                    te_tiles, _ = emit_te_dma(pair + 1, chain=False)
                    meganxt = appool.tile([128, NG * 512], F32, tag="ap",
                                          name="ap_mega")
                    emit_ap_chunk(meganxt, te_tiles, 0)
                    emit_ap_chunk(meganxt, te_tiles, 1)
                    q0 = emit_sub_m(pair, 0)
                    emit_ap_chunk(meganxt, te_tiles, 2)
                    emit_sub_h(pair, 0, q0)
                    emit_ap_chunk(meganxt, te_tiles, 3)
                    q1 = emit_sub_m(pair, 1)
                    emit_ap_chunk(meganxt, te_tiles, 4)
                    emit_ap_chunk(meganxt, te_tiles, 5)
                    emit_sub_h(pair, 1, q1)
                    mega = meganxt
                else:
                    q0 = emit_sub_m(pair, 0)
                    emit_sub_h(pair, 0, q0)
                    q1 = emit_sub_m(pair, 1)
                    emit_sub_h(pair, 1, q1)

            # ---- final eviction: out = o_psum / T + b2 ----
            out_sb = state.tile([L, R], F32, tag="out_sb", name="out_sb")
            nc.scalar.activation(
                out_sb[:], o_psum[:],
                mybir.ActivationFunctionType.Identity,
                bias=ev_bias, scale=1.0 / T,
            )
            nc.sync.dma_start(out[:], out_sb[:])

    return nc


def _f8(a):
    """fp32 -> TRN fp8e4 (IEEE e4m3, max 240) with clip."""
    return np.clip(a, -240.0, 240.0).astype(ml_dtypes.float8_e4m3)


def prep_in_maps(inputs):
    """Host-side shard + transpose + pad + cast. Returns list of per-core dicts."""
    se = np.asarray(inputs["state_embedding"], np.float32)
    te = np.asarray(inputs["tau_embedding"], np.float32)
    Wb = np.asarray(inputs["Wb"], np.float32)
    Wa = np.asarray(inputs["Wa"], np.float32)
    W1 = np.asarray(inputs["W1"], np.float32)
    b1 = np.asarray(inputs["b1"], np.float32)
    W2 = np.asarray(inputs["W2"], np.float32)
    b2 = np.asarray(inputs["b2"], np.float32)
    bf = ml_dtypes.bfloat16

    def padk(a, kd):  # pad feature axis 0
        o = np.zeros((kd,) + a.shape[1:], a.dtype)
        o[: a.shape[0]] = a
        return o

    def swz(a, cols):  # [KDB, cols] -> [128, NKB*cols] bf16 k-major
        nk = a.shape[0] // 128
        return np.ascontiguousarray(
            a.reshape(nk, 128, cols).transpose(1, 0, 2).reshape(128, nk * cols)
            .astype(bf)
        )

    # wallA (fp8): col = k256*1024 + g*256 + kk*128 + m
    WaT64 = padk(np.ascontiguousarray(Wa.T) * WSCALE, KD2)       # [KD2, F]
    wallA = np.ascontiguousarray(
        WaT64.reshape(NK2, 2, 128, NG, 128)      # [k, kk, p, g, m]
        .transpose(2, 0, 3, 1, 4)                # [p, k, g, kk, m]
        .reshape(128, NK2 * 1024)
    )
    wallA = _f8(wallA)

    wallM_wb = swz(padk(np.ascontiguousarray(Wb.T), KDB), F)
    wallB = np.zeros((128, WB_COLS), bf)
    wallB[:, O_W1 : O_W1 + NG * H] = swz(np.ascontiguousarray(W1.T), H)
    wallB[:, O_W2 : O_W2 + NG * L] = swz(np.ascontiguousarray(W2.T), L)
    c1 = (W1.sum(axis=1) + b1).astype(np.float32)
    wallB[0, O_AUG : O_AUG + NG * 128] = (-c1).astype(bf)

    cons = np.zeros((128, 2 + T), np.float32)
    cons[:L, 0] = b2
    for t in range(T):
        cons[:, 2 + t] = 2.0 ** t

    in_maps = []
    for i in range(N_CORES):
        # teT fp8: [pair, p, k128*512 + sub*R + r] ; k128 = 2*k256+kk
        tei = te[:, i * R : (i + 1) * R, :]       # [T, R, DT]
        tei = tei.reshape(NPAIR, 2 * R, DT)
        tei_p = np.zeros((NPAIR, 2 * R, KD2), np.float32)
        tei_p[:, :, :DT] = tei
        teT = np.ascontiguousarray(
            tei_p.reshape(NPAIR, 2 * R, 2 * NK2, 128)
            .transpose(0, 3, 2, 1)                # [pair, p, k128, n]
            .reshape(NPAIR, 128, NK2 * 1024)
        )
        teT = _f8(teT)
        # seT region of wallM: [p, k*T*NB + t*NB+b] = se[t, batch, d]
        sei = se[:, i * NB : (i + 1) * NB, :]     # [T, NB, DS]
        seTt = padk(np.ascontiguousarray(sei.reshape(T * NB, DS).T), KDB)
        wallM_i = np.empty((128, WM_COLS), bf)
        wallM_i[:, O_WB : O_WB + NKB * F] = wallM_wb
        wallM_i[:, O_SE : O_SE + NKB * T * NB] = swz(seTt, T * NB)
        in_maps.append(dict(teT=teT, wallA=wallA, wallM=wallM_i,
                            wallB=wallB, cons=cons))
    return in_maps


def assemble_out(core_outs):
    """[N_CORES][L, R] -> [B, L, S]"""
    full = np.stack([np.asarray(o, np.float32) for o in core_outs], axis=0)
    full = full.reshape(N_CORES, L, NB, S).transpose(0, 2, 1, 3)
    return np.ascontiguousarray(full.reshape(B, L, S))


_NC_CACHE = {}


def get_nc():
    key = "nc"
    if key not in _NC_CACHE:
        last = None
        for _ in range(6):
            try:
                _NC_CACHE[key] = build_nc()
                break
            except Exception as e:  # rare scheduler-order race-detector trip
                last = e
        else:
            raise last
    return _NC_CACHE[key]


def run_sharded(in_maps, trace=False, **kw):
    nc = get_nc()
    if not getattr(nc, "_waits_split", False):
        _split_excess_waits(nc)
        nc._waits_split = True
    res = run_bass_kernel_spmd(
        nc, in_maps, core_ids=list(range(N_CORES)), trace=trace, **kw
    )
    return res


def kernel(**inputs):
    in_maps = prep_in_maps(inputs)
    res = run_sharded(in_maps)
    return assemble_out([res.results[i]["out"] for i in range(N_CORES)])


# revision 30
# speedup vs baseline: 1.0169x; 1.0169x over previous
"""Trainium2 Bass kernel for nn_MCQuantiles (ThreeCompNode SNN scan).

Strategy (8 NeuronCores, data-parallel over batch):
- Each core takes 8 batches x 32 samples = 256 rows of the B*S axis.
- Everything runs in "transposed space": feature dims on SBUF partitions,
  batch-rows on the free dim. All transposes/swizzles are done host-side for
  free; every DMA is a flat contiguous [128, X] block.
- The apical matmul (81% of FLOPs) runs in fp8e4 DoubleRow mode (K=256 per
  instruction, 2 MACs/cell/cycle). Wa is pre-scaled by 64 host-side to avoid
  e4m3 subnormals; the 1/64 descale folds into the DVE update constants.
  Margin analysis (sim_fp8.py): layer-2 membrane max ~0.35 vs threshold 0.5,
  so fp8 noise (~0.02) cannot flip any output spike.
- Membrane recurrences use 2^t-scaled state so each update is a single fused
  op; states are fused [128, 4R] tiles (4 feature groups side by side) so one
  DVE/GpSimd instruction covers all groups.
- Engine split: DVE does psum-sourced updates + thresholds, GpSimd does the
  SBUF-only adds/resets, ACT adds the per-partition c1 bias.
- W1/W2/basal matmuls stay bf16 (fp8 would make them LDWEIGHTS-bound).
- out accumulates in a persistent PSUM bank over all T, evicted once with
  scale 1/T + bias b2.
"""
import numpy as np
import ml_dtypes

import bass_rust
import concourse.bass as bass
import concourse.mybir as mybir
from concourse.bass_utils import run_bass_kernel_spmd
from concourse.tile import TileContext
from concourse.tile_rust import add_dep_helper

# ----- problem constants (hardcoded per contract) -----
T, B, S = 8, 64, 32
DS = DT = 3136
F = H = 512
L = 18
N_CORES = 8
NB = B // N_CORES              # 8 batches per core
R = NB * S                     # 256 rows per core
NPAIR = T // 2                 # 4 step pairs
NG = F // 128                  # 4 f-tiles (= h-tiles)
WSCALE = 64.0                  # host-side fp8 weight scale (2^6, exact)

# apical path: fp8 DoubleRow, k-tiles of 256 (3136 -> 3328 = 13*256)
NK2 = 13
KD2 = NK2 * 256
# basal path: bf16, k-tiles of 128 (3136 -> 3200 = 25*128)
NKB = 25
KDB = NKB * 128

# column offsets inside wallM [128, *] (bf16)
O_WB = 0                       # basal weights, NKB*F cols
O_SE = O_WB + NKB * F          # state embeddings, NKB*T*NB cols
WM_COLS = O_SE + NKB * T * NB
# wallB (bf16): W1.T, W2.T, then the -c1 aug row (K=1 fifth W1 k-tile)
O_W1 = 0
O_W2 = O_W1 + NG * H
O_AUG = O_W2 + NG * L
WB_COLS = O_AUG + NG * 128

F32 = mybir.dt.float32
BF16 = mybir.dt.bfloat16
F8 = mybir.dt.float8e4
OP = mybir.AluOpType
DR = mybir.MatmulPerfMode.DoubleRow

# apical DMA chunks, in k256 units (total NK2=13); chunk 0 small so the
# first matmul can start early; chunks 2/3 sized ~3.2us of PE work so the
# interleave filler covers the ~3.3us membrane-chain latency
CHUNKS2 = [1, 2, 2, 2, 3, 3]
CH2_OFF = [0, 1, 3, 5, 7, 10]
NCHUNK = len(CHUNKS2)


def _patch_tile_drain():
    """This walrus build allows a single sync-wait per TPB_CTRL Drain; Tile's
    kernel-tail drain attaches one wait per active logical proc. Split them
    across a chain of drains."""
    def _patched(self, tick_clock, wait_clock):
        nc = self.nc
        drain_inst = nc.sync.drain()
        wait_clock.add_sem_waits(
            drain_inst.ins, bass_rust.ScopedClock({None: tick_clock.global_clock})
        )
        si = drain_inst.ins.sync_info
        if si is not None and len(si.on_wait) > 1:
            waits = list(si.on_wait)
            drain_inst.ins.sync_info = mybir.SyncInfo(
                on_wait=waits[:1], on_update=list(si.on_update)
            )
            for w in waits[1:]:
                extra = nc.sync.drain()
                extra.ins.sync_info = mybir.SyncInfo(on_wait=[w], on_update=[])
        nc.all_engine_barrier()
        popped = nc._tile_sem_poison_stack.pop()
        assert popped is self._sem_poison
        nc.clear_and_free_semaphores(list(self.sems.allocated().values()))
        nc.all_engine_barrier()

    TileContext._drain_and_barrier = _patched


def _split_excess_waits(nc, limit=1):
    """Walrus here rejects instructions carrying more than ~1 sync-wait. Move
    excess waits onto same-engine NoOps inserted just before the instruction."""
    for fn in nc.m.functions:
        for bb in fn.blocks:
            new = []
            changed = False
            for inst in bb.instructions:
                si = getattr(inst, "sync_info", None)
                ow = list(si.on_wait) if si is not None and si.on_wait else []
                if len(ow) > limit:
                    extra = ow[limit:]
                    for j in range(0, len(extra), limit):
                        nop = mybir.InstNoOp(
                            name=f"{inst.name}-ws{j}", ins=[], outs=[]
                        )
                        nop.engine = inst.engine
                        nop.sync_info = mybir.SyncInfo(
                            on_wait=extra[j : j + limit], on_update=[]
                        )
                        new.append(nop)
                    inst.sync_info = mybir.SyncInfo(
                        on_wait=ow[:limit], on_update=list(si.on_update)
                    )
                    changed = True
                new.append(inst)
            if changed:
                try:
                    bb.instructions[:] = new
                except TypeError:
                    bb.instructions = new


def build_nc():
    _patch_tile_drain()
    nc = bass.Bass()

    teT = nc.declare_dram_parameter("teT", [NPAIR, 128, NK2 * 1024], F8, isOutput=False)
    wallA = nc.declare_dram_parameter("wallA", [128, NK2 * 1024], F8, isOutput=False)
    wallM = nc.declare_dram_parameter("wallM", [128, WM_COLS], BF16, isOutput=False)
    wallB = nc.declare_dram_parameter("wallB", [128, WB_COLS], BF16, isOutput=False)
    # cons: [0] = c2+b2 eviction bias (rows < L); [1] = 0.0; [2+t] = 2^t
    cons = nc.declare_dram_parameter("cons", [128, 2 + T], F32, isOutput=False)
    out = nc.declare_dram_parameter("out", [L, R], F32, isOutput=True)

    with TileContext(nc) as tc:
        with (
            tc.tile_pool(name="wpool", bufs=1) as wpool,
            tc.tile_pool(name="tepool", bufs=2) as tepool,
            tc.tile_pool(name="state", bufs=1) as state,
            tc.tile_pool(name="qpool", bufs=2) as qpool,
            tc.tile_pool(name="appool", bufs=1, space="PSUM") as appool,
            tc.tile_pool(name="hqpool", bufs=1, space="PSUM") as hqpool,
            tc.tile_pool(name="bopool", bufs=1, space="PSUM") as bopool,
        ):
            # ---- resident weights/constants ----
            wallA_c = []
            prev = None
            for c in range(NCHUNK):
                ck = wpool.tile(
                    [128, CHUNKS2[c] * 1024], F8, tag=f"wallA{c}", name=f"wa_ck{c}"
                )
                wallA_c.append(ck)
                d = nc.sync.dma_start(
                    ck[:],
                    wallA[:, CH2_OFF[c] * 1024 : (CH2_OFF[c] + CHUNKS2[c]) * 1024],
                )
                if prev is not None:
                    add_dep_helper(d.ins, prev.ins, reason="serialize wallA chunks")
                prev = d
            wallM_sb = wpool.tile([128, WM_COLS], BF16, tag="wallM", name="wallM_sb")
            wallB_sb = wpool.tile([128, WB_COLS], BF16, tag="wallB", name="wallB_sb")
            cons_sb = wpool.tile([128, 2 + T], F32, tag="cons", name="cons_sb")

            def waT8(c, kk, g):
                """fp8 DoubleRow lhsT [128, 2, 128] for local k256 kk, group g."""
                base = kk * 1024 + g * 256
                return wallA_c[c][:, base : base + 256].rearrange(
                    "p (two f) -> p two f", two=2
                )

            def wbT(k, g):
                return wallM_sb[:, O_WB + k * F + g * 128 : O_WB + k * F + (g + 1) * 128]

            def seT(k):
                return wallM_sb[:, O_SE + k * T * NB : O_SE + (k + 1) * T * NB]

            def w1T(k, g):
                return wallB_sb[:, O_W1 + k * H + g * 128 : O_W1 + k * H + (g + 1) * 128]

            def w2T(k):
                return wallB_sb[:, O_W2 + k * L : O_W2 + (k + 1) * L]

            def waug(g):  # [1, 128] fifth-k-tile row: -c1 slice
                return wallB_sb[0:1, O_AUG + g * 128 : O_AUG + (g + 1) * 128]

            ev_bias = cons_sb[0:L, 0:1]          # b2
            zero_ap = cons_sb[:, 1:2]

            def th2_ap(t):                       # 2^t column
                return cons_sb[:, 2 + t : 3 + t]

            # ---- fused state tiles ([128, 4R]: 4 groups side by side) ----
            A = [state.tile([128, NG * R], BF16, tag=f"A{p}", name=f"A{p}")
                 for p in range(2)]          # alpha ping-pong (even/odd t)
            M = state.tile([128, NG * R], BF16, tag="M", name="M")
            ML = state.tile([128, NG * R], BF16, tag="ML", name="ML")
            Bsc = state.tile([128, T * NG * NB], BF16, tag="Bsc", name="Bsc")
            ones = state.tile([1, R], BF16, tag="ones", name="ones")
            nc.vector.memset(ones[:], 1.0)

            o_psum = bopool.tile([L, R], F32, tag="o", name="o_psum")

            def A3(p):
                return A[p].rearrange("p (g r) -> p g r", g=NG)

            def B_bc(t, g):
                base = t * NG * NB + g * NB
                return (
                    Bsc[:, base : base + NB]
                    .unsqueeze(2)
                    .broadcast_to([128, NB, S])
                )

            def M_bs(g):
                return M[:, g * R : (g + 1) * R].rearrange(
                    "p (b s) -> p b s", s=S
                )

            # ---- emission helpers ----
            def emit_te_dma(pair, chain):
                tiles = []
                prev = None
                for c in range(NCHUNK):
                    tck = tepool.tile(
                        [128, CHUNKS2[c] * 1024], F8, tag=f"te{c}", name=f"te_ck{c}"
                    )
                    tiles.append(tck)
                    d = nc.sync.dma_start(
                        tck[:],
                        teT[pair][:, CH2_OFF[c] * 1024
                                  : (CH2_OFF[c] + CHUNKS2[c]) * 1024],
                    )
                    if prev is not None and chain:
                        add_dep_helper(d.ins, prev.ins,
                                       reason="serialize te chunk DMAs")
                    prev = d
                return tiles, prev

            def emit_ap_chunk(mega, te_tiles, c):
                for g in range(NG):
                    for kk in range(CHUNKS2[c]):
                        k = CH2_OFF[c] + kk
                        rhs = te_tiles[c][:, kk * 1024 : (kk + 1) * 1024].rearrange(
                            "p (two n) -> p two n", two=2
                        )
                        nc.tensor.matmul(
                            mega[:, g * 512 : (g + 1) * 512],
                            lhsT=waT8(c, kk, g),
                            rhs=rhs,
                            start=(k == 0),
                            stop=(k == NK2 - 1),
                            perf_mode=DR,
                        )

            def emit_a_updates(mega, pair):
                """alpha_t = alpha_{t-1} + (2^{t-1}/64) * ap64_t, fused over
                groups; ping-pong A[t%2]."""
                mega3 = mega.rearrange("p (g x) -> p g x", g=NG)
                for sub in range(2):
                    t = 2 * pair + sub
                    s_t = float(2 ** (t - 1)) / WSCALE
                    ap3 = mega3[:, :, sub * R : (sub + 1) * R]
                    if t == 0:
                        nc.vector.tensor_scalar(A3(0), ap3, s_t, None, OP.mult)
                    else:
                        nc.vector.scalar_tensor_tensor(
                            A3(t % 2), ap3, s_t, A3(1 - t % 2), OP.mult, OP.add
                        )

            def emit_basal():
                bs_ps = bopool.tile([128, NG * T * NB], F32, tag="bs", name="bs_ps")
                for g in range(NG):
                    for k in range(NKB):
                        nc.tensor.matmul(
                            bs_ps[:, g * T * NB : (g + 1) * T * NB],
                            lhsT=wbT(k, g),
                            rhs=seT(k),
                            start=(k == 0),
                            stop=(k == NKB - 1),
                        )
                ps3 = bs_ps.rearrange("p (g tb) -> p g tb", g=NG)
                for t in range(T):
                    src = ps3[:, :, t * NB : (t + 1) * NB]
                    dst = Bsc[:, t * NG * NB : (t + 1) * NG * NB].rearrange(
                        "p (g b) -> p g b", g=NG
                    )
                    if t == 0:
                        nc.vector.tensor_scalar(dst, src, 0.5, None, OP.mult)
                    else:
                        prv = Bsc[:, (t - 1) * NG * NB : t * NG * NB].rearrange(
                            "p (g b) -> p g b", g=NG
                        )
                        nc.vector.scalar_tensor_tensor(
                            dst, src, float(2 ** (t - 1)), prv, OP.mult, OP.add
                        )

            def emit_sub_m(pair, sub):
                """Membrane phase: DVE-only (M update, q, reset)."""
                t = 2 * pair + sub
                th_t = float(2 ** (t + 1))
                Ax = A[t % 2]
                # --- membrane: M = M_post + A_t + B_t (per-group DVE ops) ---
                for g in range(NG):
                    sl = slice(g * R, (g + 1) * R)
                    if t == 0:
                        nc.vector.tensor_tensor(
                            M_bs(g), A[0][:, sl].rearrange("p (b s) -> p b s", s=S),
                            B_bc(0, g), OP.add,
                        )
                    else:
                        nc.vector.tensor_tensor(
                            M[:, sl], M[:, sl], Ax[:, sl], OP.add
                        )
                        nc.vector.tensor_tensor(
                            M_bs(g), B_bc(t, g), M_bs(g), OP.add
                        )
                # spikes (DVE): q = NOT(spike) = (M <= th)
                qg = qpool.tile([128, NG * R], BF16, tag="q", name="qg")
                nc.vector.tensor_scalar(qg[:], M[:], th_t, None, OP.is_le)
                if t < T - 1:
                    # hard reset (DVE): M *= q  (dead at the last step)
                    nc.vector.tensor_tensor(M[:], M[:], qg[:], OP.mult)
                return qg

            def emit_sub_h(pair, sub, qg):
                """Matmul + LIF phase (PE/ACT/DVE)."""
                t = 2 * pair + sub
                sc_t = float(2 ** t)
                # --- layer 1: hq = q @ W1.T - c1 (bf16, fused psum); the -c1
                # aug matmul seeds each group (start=True) so it runs while
                # the PE would otherwise wait for q ---
                hq = hqpool.tile([128, NG * R], F32, tag="hq", name="hq")
                # hq spans 2 psum banks (g0,g1 | g2,g3): only one pending
                # accumulation group per bank, so seed bank-disjoint pairs.
                for gs in ((0, 2), (1, 3)):
                    for g in gs:
                        nc.tensor.matmul(
                            hq[:, g * R : (g + 1) * R],
                            lhsT=waug(g), rhs=ones[:],
                            start=True, stop=False,
                        )
                    for g in gs:
                        for k in range(NG):
                            nc.tensor.matmul(
                                hq[:, g * R : (g + 1) * R],
                                lhsT=w1T(k, g),
                                rhs=qg[:, k * R : (k + 1) * R],
                                start=False,
                                stop=(k == NG - 1),
                            )

                # --- LIF: ML = ML - 2^t * hq' ---
                if t == 0:
                    nc.vector.tensor_scalar(ML[:], hq[:], -1.0, None, OP.mult)
                else:
                    nc.vector.scalar_tensor_tensor(
                        ML[:], hq[:], -sc_t, ML[:], OP.mult, OP.add
                    )
                # qs = Sign(2^t - ML) in {-1,0,1}; qL = Relu(qs) = NOT(spike2);
                # sp2 = Relu(-qs). sp2 (not a qL-fold) keeps out EXACTLY zero
                # when no spike fires.
                qs = qpool.tile([128, NG * R], BF16, tag="qs", name="qs")
                nc.scalar.activation(
                    qs[:], ML[:], mybir.ActivationFunctionType.Sign,
                    bias=th2_ap(t), scale=-1.0,
                )
                spg = qpool.tile([128, NG * R], BF16, tag="sp2", name="spg")
                nc.scalar.activation(
                    spg[:], qs[:], mybir.ActivationFunctionType.Relu,
                    bias=zero_ap, scale=-1.0,
                )
                if t < T - 1:
                    # qL + LIF reset are dead at the last step
                    qL = qpool.tile([128, NG * R], BF16, tag="qL", name="qL")
                    nc.scalar.activation(
                        qL[:], qs[:], mybir.ActivationFunctionType.Relu,
                        bias=zero_ap, scale=1.0,
                    )
                    nc.vector.tensor_tensor(ML[:], ML[:], qL[:], OP.mult)

                # --- layer 2: out += sp2 @ W2.T (persistent psum) ---
                for k in range(NG):
                    nc.tensor.matmul(
                        o_psum[:],
                        lhsT=w2T(k),
                        rhs=spg[:, k * R : (k + 1) * R],
                        start=(t == 0 and k == 0),
                        stop=(t == T - 1 and k == NG - 1),
                    )

            # ---- prologue: pair 0 load + apical ----
            te_tiles, last_te_dma = emit_te_dma(0, chain=True)
            # HAM warmup: ~4us of dummy matmuls on the ones tile during the
            # otherwise-idle input-DMA window, so real matmuls start at 2.4GHz
            warm = hqpool.tile([128, NG * R], F32, tag="hq", name="warmup")
            for _ in range(12):
                nc.tensor.matmul(
                    warm[:, 0:R], lhsT=ones[0:1, 0:128], rhs=ones[:],
                    start=True, stop=True,
                )
            mega = appool.tile([128, NG * 512], F32, tag="ap", name="ap_mega")
            for c in range(NCHUNK):
                emit_ap_chunk(mega, te_tiles, c)
            dM = nc.sync.dma_start(wallM_sb[:], wallM[:])
            add_dep_helper(dM.ins, last_te_dma.ins, reason="wallM after te0 chain")
            dB = nc.sync.dma_start(wallB_sb[:], wallB[:])
            add_dep_helper(dB.ins, dM.ins, reason="wallB after wallM")
            dC = nc.sync.dma_start(cons_sb[:], cons[:])
            add_dep_helper(dC.ins, dM.ins, reason="cons after wallM")
            emit_basal()

            # ---- software-pipelined main loop ----
            for pair in range(NPAIR):
                emit_a_updates(mega, pair)
                if pair + 1 < NPAIR:
                    te_tiles, _ = emit_te_dma(pair + 1, chain=False)
                    meganxt = appool.tile([128, NG * 512], F32, tag="ap",
                                          name="ap_mega")
                    emit_ap_chunk(meganxt, te_tiles, 0)
                    emit_ap_chunk(meganxt, te_tiles, 1)
                    q0 = emit_sub_m(pair, 0)
                    emit_ap_chunk(meganxt, te_tiles, 2)
                    emit_sub_h(pair, 0, q0)
                    emit_ap_chunk(meganxt, te_tiles, 3)
                    q1 = emit_sub_m(pair, 1)
                    emit_ap_chunk(meganxt, te_tiles, 4)
                    emit_ap_chunk(meganxt, te_tiles, 5)
                    emit_sub_h(pair, 1, q1)
                    mega = meganxt
                else:
                    q0 = emit_sub_m(pair, 0)
                    emit_sub_h(pair, 0, q0)
                    q1 = emit_sub_m(pair, 1)
                    emit_sub_h(pair, 1, q1)

            # ---- final eviction: out = o_psum / T + b2 ----
            out_sb = state.tile([L, R], F32, tag="out_sb", name="out_sb")
            nc.scalar.activation(
                out_sb[:], o_psum[:],
                mybir.ActivationFunctionType.Identity,
                bias=ev_bias, scale=1.0 / T,
            )
            nc.sync.dma_start(out[:], out_sb[:])

    return nc


def _f8(a):
    """fp32 -> TRN fp8e4 (IEEE e4m3, max 240) with clip."""
    return np.clip(a, -240.0, 240.0).astype(ml_dtypes.float8_e4m3)


def prep_in_maps(inputs):
    """Host-side shard + transpose + pad + cast. Returns list of per-core dicts."""
    se = np.asarray(inputs["state_embedding"], np.float32)
    te = np.asarray(inputs["tau_embedding"], np.float32)
    Wb = np.asarray(inputs["Wb"], np.float32)
    Wa = np.asarray(inputs["Wa"], np.float32)
    W1 = np.asarray(inputs["W1"], np.float32)
    b1 = np.asarray(inputs["b1"], np.float32)
    W2 = np.asarray(inputs["W2"], np.float32)
    b2 = np.asarray(inputs["b2"], np.float32)
    bf = ml_dtypes.bfloat16

    def padk(a, kd):  # pad feature axis 0
        o = np.zeros((kd,) + a.shape[1:], a.dtype)
        o[: a.shape[0]] = a
        return o

    def swz(a, cols):  # [KDB, cols] -> [128, NKB*cols] bf16 k-major
        nk = a.shape[0] // 128
        return np.ascontiguousarray(
            a.reshape(nk, 128, cols).transpose(1, 0, 2).reshape(128, nk * cols)
            .astype(bf)
        )

    # wallA (fp8): col = k256*1024 + g*256 + kk*128 + m
    WaT64 = padk(np.ascontiguousarray(Wa.T) * WSCALE, KD2)       # [KD2, F]
    wallA = np.ascontiguousarray(
        WaT64.reshape(NK2, 2, 128, NG, 128)      # [k, kk, p, g, m]
        .transpose(2, 0, 3, 1, 4)                # [p, k, g, kk, m]
        .reshape(128, NK2 * 1024)
    )
    wallA = _f8(wallA)

    wallM_wb = swz(padk(np.ascontiguousarray(Wb.T), KDB), F)
    wallB = np.zeros((128, WB_COLS), bf)
    wallB[:, O_W1 : O_W1 + NG * H] = swz(np.ascontiguousarray(W1.T), H)
    wallB[:, O_W2 : O_W2 + NG * L] = swz(np.ascontiguousarray(W2.T), L)
    c1 = (W1.sum(axis=1) + b1).astype(np.float32)
    wallB[0, O_AUG : O_AUG + NG * 128] = (-c1).astype(bf)

    cons = np.zeros((128, 2 + T), np.float32)
    cons[:L, 0] = b2
    for t in range(T):
        cons[:, 2 + t] = 2.0 ** t

    in_maps = []
    for i in range(N_CORES):
        # teT fp8: [pair, p, k128*512 + sub*R + r] ; k128 = 2*k256+kk
        tei = te[:, i * R : (i + 1) * R, :]       # [T, R, DT]
        tei = tei.reshape(NPAIR, 2 * R, DT)
        tei_p = np.zeros((NPAIR, 2 * R, KD2), np.float32)
        tei_p[:, :, :DT] = tei
        teT = np.ascontiguousarray(
            tei_p.reshape(NPAIR, 2 * R, 2 * NK2, 128)
            .transpose(0, 3, 2, 1)                # [pair, p, k128, n]
            .reshape(NPAIR, 128, NK2 * 1024)
        )
        teT = _f8(teT)
        # seT region of wallM: [p, k*T*NB + t*NB+b] = se[t, batch, d]
        sei = se[:, i * NB : (i + 1) * NB, :]     # [T, NB, DS]
        seTt = padk(np.ascontiguousarray(sei.reshape(T * NB, DS).T), KDB)
        wallM_i = np.empty((128, WM_COLS), bf)
        wallM_i[:, O_WB : O_WB + NKB * F] = wallM_wb
        wallM_i[:, O_SE : O_SE + NKB * T * NB] = swz(seTt, T * NB)
        in_maps.append(dict(teT=teT, wallA=wallA, wallM=wallM_i,
                            wallB=wallB, cons=cons))
    return in_maps


def assemble_out(core_outs):
    """[N_CORES][L, R] -> [B, L, S]"""
    full = np.stack([np.asarray(o, np.float32) for o in core_outs], axis=0)
    full = full.reshape(N_CORES, L, NB, S).transpose(0, 2, 1, 3)
    return np.ascontiguousarray(full.reshape(B, L, S))


_NC_CACHE = {}


def get_nc():
    key = "nc"
    if key not in _NC_CACHE:
        last = None
        for _ in range(6):
            try:
                _NC_CACHE[key] = build_nc()
                break
            except Exception as e:  # rare scheduler-order race-detector trip
                last = e
        else:
            raise last
    return _NC_CACHE[key]


def run_sharded(in_maps, trace=False, **kw):
    nc = get_nc()
    if not getattr(nc, "_waits_split", False):
        _split_excess_waits(nc)
        nc._waits_split = True
    res = run_bass_kernel_spmd(
        nc, in_maps, core_ids=list(range(N_CORES)), trace=trace, **kw
    )
    return res


def kernel(**inputs):
    in_maps = prep_in_maps(inputs)
    res = run_sharded(in_maps)
    return assemble_out([res.results[i]["out"] for i in range(N_CORES)])


# revision 35
# speedup vs baseline: 1.0646x; 1.0469x over previous
"""Trainium2 Bass kernel for nn_MCQuantiles (ThreeCompNode SNN scan).

Strategy (8 NeuronCores, data-parallel over batch):
- Each core takes 8 batches x 32 samples = 256 rows of the B*S axis.
- Everything runs in "transposed space": feature dims on SBUF partitions,
  batch-rows on the free dim. All transposes/swizzles are done host-side for
  free; every DMA is a flat contiguous [128, X] block.
- The apical matmul (81% of FLOPs) runs in fp8e4 DoubleRow mode (K=256 per
  instruction, 2 MACs/cell/cycle). Wa is pre-scaled by 64 host-side to avoid
  e4m3 subnormals; the 1/64 descale folds into the DVE update constants.
  Margin analysis (sim_fp8.py): layer-2 membrane max ~0.35 vs threshold 0.5,
  so fp8 noise (~0.02) cannot flip any output spike.
- Membrane recurrences use 2^t-scaled state so each update is a single fused
  op; states are fused [128, 4R] tiles (4 feature groups side by side) so one
  DVE/GpSimd instruction covers all groups.
- Engine split: DVE does psum-sourced updates + thresholds, GpSimd does the
  SBUF-only adds/resets, ACT adds the per-partition c1 bias.
- W1/W2/basal matmuls stay bf16 (fp8 would make them LDWEIGHTS-bound).
- out accumulates in a persistent PSUM bank over all T, evicted once with
  scale 1/T + bias b2.
"""
import numpy as np
import ml_dtypes

import bass_rust
import concourse.bass as bass
import concourse.mybir as mybir
from concourse.bass_utils import run_bass_kernel_spmd
from concourse.tile import TileContext
from concourse.tile_rust import add_dep_helper

# ----- problem constants (hardcoded per contract) -----
T, B, S = 8, 64, 32
DS = DT = 3136
F = H = 512
L = 18
N_CORES = 8
NB = B // N_CORES              # 8 batches per core
R = NB * S                     # 256 rows per core
NPAIR = T // 2                 # 4 step pairs
NG = F // 128                  # 4 f-tiles (= h-tiles)
WSCALE = 64.0                  # host-side fp8 weight scale (2^6, exact)

# apical path: fp8 DoubleRow, k-tiles of 256 (3136 -> 3328 = 13*256)
NK2 = 13
KD2 = NK2 * 256
# basal path: bf16, k-tiles of 128 (3136 -> 3200 = 25*128)
NKB = 25
KDB = NKB * 128

# column offsets inside wallM [128, *] (bf16)
O_WB = 0                       # basal weights, NKB*F cols
O_SE = O_WB + NKB * F          # state embeddings, NKB*T*NB cols
WM_COLS = O_SE + NKB * T * NB
# wallB (bf16): W1.T, W2.T, then the -c1 aug row (K=1 fifth W1 k-tile)
O_W1 = 0
O_W2 = O_W1 + NG * H
O_AUG = O_W2 + NG * L
WB_COLS = O_AUG + NG * 128

F32 = mybir.dt.float32
BF16 = mybir.dt.bfloat16
F8 = mybir.dt.float8e4
OP = mybir.AluOpType
DR = mybir.MatmulPerfMode.DoubleRow

# apical DMA chunks, in k256 units (total NK2=13); chunk 0 small so the
# first matmul can start early; chunks 2/3 sized ~3.2us of PE work so the
# interleave filler covers the ~3.3us membrane-chain latency
CHUNKS2 = [1, 2, 2, 2, 3, 3]
CH2_OFF = [0, 1, 3, 5, 7, 10]
NCHUNK = len(CHUNKS2)


def _patch_tile_drain():
    """This walrus build allows a single sync-wait per TPB_CTRL Drain; Tile's
    kernel-tail drain attaches one wait per active logical proc. Split them
    across a chain of drains."""
    def _patched(self, tick_clock, wait_clock):
        nc = self.nc
        drain_inst = nc.sync.drain()
        wait_clock.add_sem_waits(
            drain_inst.ins, bass_rust.ScopedClock({None: tick_clock.global_clock})
        )
        si = drain_inst.ins.sync_info
        if si is not None and len(si.on_wait) > 1:
            waits = list(si.on_wait)
            drain_inst.ins.sync_info = mybir.SyncInfo(
                on_wait=waits[:1], on_update=list(si.on_update)
            )
            for w in waits[1:]:
                extra = nc.sync.drain()
                extra.ins.sync_info = mybir.SyncInfo(on_wait=[w], on_update=[])
        nc.all_engine_barrier()
        popped = nc._tile_sem_poison_stack.pop()
        assert popped is self._sem_poison
        nc.clear_and_free_semaphores(list(self.sems.allocated().values()))
        nc.all_engine_barrier()

    TileContext._drain_and_barrier = _patched


def _split_excess_waits(nc, limit=1):
    """Walrus here rejects instructions carrying more than ~1 sync-wait. Move
    excess waits onto same-engine NoOps inserted just before the instruction."""
    for fn in nc.m.functions:
        for bb in fn.blocks:
            new = []
            changed = False
            for inst in bb.instructions:
                si = getattr(inst, "sync_info", None)
                ow = list(si.on_wait) if si is not None and si.on_wait else []
                if len(ow) > limit:
                    extra = ow[limit:]
                    for j in range(0, len(extra), limit):
                        nop = mybir.InstNoOp(
                            name=f"{inst.name}-ws{j}", ins=[], outs=[]
                        )
                        nop.engine = inst.engine
                        nop.sync_info = mybir.SyncInfo(
                            on_wait=extra[j : j + limit], on_update=[]
                        )
                        new.append(nop)
                    inst.sync_info = mybir.SyncInfo(
                        on_wait=ow[:limit], on_update=list(si.on_update)
                    )
                    changed = True
                new.append(inst)
            if changed:
                try:
                    bb.instructions[:] = new
                except TypeError:
                    bb.instructions = new


def build_nc():
    _patch_tile_drain()
    nc = bass.Bass()

    teT = nc.declare_dram_parameter("teT", [NPAIR, 128, NK2 * 1024], F8, isOutput=False)
    wallA = nc.declare_dram_parameter("wallA", [128, NK2 * 1024], F8, isOutput=False)
    wallM = nc.declare_dram_parameter("wallM", [128, WM_COLS], BF16, isOutput=False)
    wallB = nc.declare_dram_parameter("wallB", [128, WB_COLS], BF16, isOutput=False)
    # cons: [0] = c2+b2 eviction bias (rows < L); [1] = 0.0; [2+t] = 2^t
    cons = nc.declare_dram_parameter("cons", [128, 2 + T], F32, isOutput=False)
    out = nc.declare_dram_parameter("out", [L, R], F32, isOutput=True)

    with TileContext(nc) as tc:
        with (
            tc.tile_pool(name="wpool", bufs=1) as wpool,
            tc.tile_pool(name="tepool", bufs=2) as tepool,
            tc.tile_pool(name="state", bufs=1) as state,
            tc.tile_pool(name="qpool", bufs=2) as qpool,
            tc.tile_pool(name="appool", bufs=1, space="PSUM") as appool,
            tc.tile_pool(name="hqpool", bufs=1, space="PSUM") as hqpool,
            tc.tile_pool(name="bopool", bufs=1, space="PSUM") as bopool,
        ):
            # ---- resident weights/constants ----
            wallA_c = []
            prev = None
            for c in range(NCHUNK):
                ck = wpool.tile(
                    [128, CHUNKS2[c] * 1024], F8, tag=f"wallA{c}", name=f"wa_ck{c}"
                )
                wallA_c.append(ck)
                d = nc.sync.dma_start(
                    ck[:],
                    wallA[:, CH2_OFF[c] * 1024 : (CH2_OFF[c] + CHUNKS2[c]) * 1024],
                )
                if prev is not None:
                    add_dep_helper(d.ins, prev.ins, reason="serialize wallA chunks")
                prev = d
            wallM_sb = wpool.tile([128, WM_COLS], BF16, tag="wallM", name="wallM_sb")
            wallB_sb = wpool.tile([128, WB_COLS], BF16, tag="wallB", name="wallB_sb")
            cons_sb = wpool.tile([128, 2 + T], F32, tag="cons", name="cons_sb")

            def waT8(c, kk, g):
                """fp8 DoubleRow lhsT [128, 2, 128] for local k256 kk, group g."""
                base = kk * 1024 + g * 256
                return wallA_c[c][:, base : base + 256].rearrange(
                    "p (two f) -> p two f", two=2
                )

            def wbT(k, g):
                return wallM_sb[:, O_WB + k * F + g * 128 : O_WB + k * F + (g + 1) * 128]

            def seT(k):
                return wallM_sb[:, O_SE + k * T * NB : O_SE + (k + 1) * T * NB]

            def w1T(k, g):
                return wallB_sb[:, O_W1 + k * H + g * 128 : O_W1 + k * H + (g + 1) * 128]

            def w2T(k):
                return wallB_sb[:, O_W2 + k * L : O_W2 + (k + 1) * L]

            def waug(g):  # [1, 128] fifth-k-tile row: -c1 slice
                return wallB_sb[0:1, O_AUG + g * 128 : O_AUG + (g + 1) * 128]

            ev_bias = cons_sb[0:L, 0:1]          # b2
            zero_ap = cons_sb[:, 1:2]

            def th2_ap(t):                       # 2^t column
                return cons_sb[:, 2 + t : 3 + t]

            # ---- fused state tiles ([128, 4R]: 4 groups side by side) ----
            A = [state.tile([128, NG * R], BF16, tag=f"A{p}", name=f"A{p}")
                 for p in range(2)]          # alpha ping-pong (even/odd t)
            M = state.tile([128, NG * R], BF16, tag="M", name="M")
            ML = state.tile([128, NG * R], BF16, tag="ML", name="ML")
            Bsc = state.tile([128, T * NG * NB], BF16, tag="Bsc", name="Bsc")
            ones = state.tile([1, R], BF16, tag="ones", name="ones")
            nc.vector.memset(ones[:], 1.0)

            o_psum = bopool.tile([L, R], F32, tag="o", name="o_psum")

            def A3(p):
                return A[p].rearrange("p (g r) -> p g r", g=NG)

            def B_bc(t, g):
                base = t * NG * NB + g * NB
                return (
                    Bsc[:, base : base + NB]
                    .unsqueeze(2)
                    .broadcast_to([128, NB, S])
                )

            def M_bs(g):
                return M[:, g * R : (g + 1) * R].rearrange(
                    "p (b s) -> p b s", s=S
                )

            # ---- emission helpers ----
            def emit_te_dma(pair, chain):
                tiles = []
                prev = None
                for c in range(NCHUNK):
                    tck = tepool.tile(
                        [128, CHUNKS2[c] * 1024], F8, tag=f"te{c}", name=f"te_ck{c}"
                    )
                    tiles.append(tck)
                    d = nc.sync.dma_start(
                        tck[:],
                        teT[pair][:, CH2_OFF[c] * 1024
                                  : (CH2_OFF[c] + CHUNKS2[c]) * 1024],
                    )
                    if prev is not None and chain:
                        add_dep_helper(d.ins, prev.ins,
                                       reason="serialize te chunk DMAs")
                    prev = d
                return tiles, prev

            def emit_ap_chunk(mega, te_tiles, c):
                for g in range(NG):
                    for kk in range(CHUNKS2[c]):
                        k = CH2_OFF[c] + kk
                        rhs = te_tiles[c][:, kk * 1024 : (kk + 1) * 1024].rearrange(
                            "p (two n) -> p two n", two=2
                        )
                        nc.tensor.matmul(
                            mega[:, g * 512 : (g + 1) * 512],
                            lhsT=waT8(c, kk, g),
                            rhs=rhs,
                            start=(k == 0),
                            stop=(k == NK2 - 1),
                            perf_mode=DR,
                        )

            def emit_a_updates(mega, pair):
                """alpha_t = alpha_{t-1} + (2^{t-1}/64) * ap64_t, fused over
                groups; ping-pong A[t%2]."""
                mega3 = mega.rearrange("p (g x) -> p g x", g=NG)
                for sub in range(2):
                    t = 2 * pair + sub
                    s_t = float(2 ** (t - 1)) / WSCALE
                    ap3 = mega3[:, :, sub * R : (sub + 1) * R]
                    if t == 0:
                        nc.vector.tensor_scalar(A3(0), ap3, s_t, None, OP.mult)
                    else:
                        nc.vector.scalar_tensor_tensor(
                            A3(t % 2), ap3, s_t, A3(1 - t % 2), OP.mult, OP.add
                        )

            def emit_basal():
                bs_ps = bopool.tile([128, NG * T * NB], F32, tag="bs", name="bs_ps")
                for g in range(NG):
                    for k in range(NKB):
                        nc.tensor.matmul(
                            bs_ps[:, g * T * NB : (g + 1) * T * NB],
                            lhsT=wbT(k, g),
                            rhs=seT(k),
                            start=(k == 0),
                            stop=(k == NKB - 1),
                        )
                ps3 = bs_ps.rearrange("p (g tb) -> p g tb", g=NG)
                for t in range(T):
                    src = ps3[:, :, t * NB : (t + 1) * NB]
                    dst = Bsc[:, t * NG * NB : (t + 1) * NG * NB].rearrange(
                        "p (g b) -> p g b", g=NG
                    )
                    if t == 0:
                        nc.vector.tensor_scalar(dst, src, 0.5, None, OP.mult)
                    else:
                        prv = Bsc[:, (t - 1) * NG * NB : t * NG * NB].rearrange(
                            "p (g b) -> p g b", g=NG
                        )
                        nc.vector.scalar_tensor_tensor(
                            dst, src, float(2 ** (t - 1)), prv, OP.mult, OP.add
                        )

            def emit_sub_m(pair, sub):
                """Membrane phase (DVE): B_t was pre-added after step t-1's
                reset, so the psum->W1 critical path is only M+=A then q."""
                t = 2 * pair + sub
                th_t = float(2 ** (t + 1))
                Ax = A[t % 2]
                for g in range(NG):
                    sl = slice(g * R, (g + 1) * R)
                    if t == 0:
                        nc.vector.tensor_tensor(
                            M_bs(g), A[0][:, sl].rearrange("p (b s) -> p b s", s=S),
                            B_bc(0, g), OP.add,
                        )
                    else:
                        nc.vector.tensor_tensor(
                            M[:, sl], M[:, sl], Ax[:, sl], OP.add
                        )
                # spikes (DVE), split in halves so W1's k=0,1 matmuls can
                # start as soon as the first half lands
                qg = qpool.tile([128, NG * R], BF16, tag="q", name="qg")
                HR = NG * R // 2
                nc.vector.tensor_scalar(qg[:, :HR], M[:, :HR], th_t, None, OP.is_le)
                nc.vector.tensor_scalar(qg[:, HR:], M[:, HR:], th_t, None, OP.is_le)
                # hard reset (DVE): M *= q
                nc.vector.tensor_tensor(M[:], M[:], qg[:], OP.mult)
                # pre-add B_{t+1} for the next step (off the critical path)
                if t + 1 < T:
                    for g in range(NG):
                        nc.vector.tensor_tensor(
                            M_bs(g), B_bc(t + 1, g), M_bs(g), OP.add
                        )
                return qg

            def emit_sub_h(pair, sub, qg):
                """Matmul + LIF phase (PE/ACT/DVE)."""
                t = 2 * pair + sub
                sc_t = float(2 ** t)
                # --- layer 1: hq = q @ W1.T - c1 (bf16, fused psum); the -c1
                # aug matmul seeds each group (start=True) so it runs while
                # the PE would otherwise wait for q ---
                hq = hqpool.tile([128, NG * R], F32, tag="hq", name="hq")
                # hq spans 2 psum banks (g0,g1 | g2,g3): only one pending
                # accumulation group per bank, so seed bank-disjoint pairs.
                for gs in ((0, 2), (1, 3)):
                    for g in gs:
                        nc.tensor.matmul(
                            hq[:, g * R : (g + 1) * R],
                            lhsT=waug(g), rhs=ones[:],
                            start=True, stop=False,
                        )
                    # k-outer: k=0,1 depend only on the first q half
                    for k in range(NG):
                        for g in gs:
                            nc.tensor.matmul(
                                hq[:, g * R : (g + 1) * R],
                                lhsT=w1T(k, g),
                                rhs=qg[:, k * R : (k + 1) * R],
                                start=False,
                                stop=(k == NG - 1),
                            )

                # --- LIF: ML = ML - 2^t * hq' ---
                if t == 0:
                    nc.vector.tensor_scalar(ML[:], hq[:], -1.0, None, OP.mult)
                else:
                    nc.vector.scalar_tensor_tensor(
                        ML[:], hq[:], -sc_t, ML[:], OP.mult, OP.add
                    )
                # qs = Sign(2^t - ML) in {-1,0,1}; qL = Relu(qs) = NOT(spike2);
                # sp2 = Relu(-qs). sp2 (not a qL-fold) keeps out EXACTLY zero
                # when no spike fires.
                qs = qpool.tile([128, NG * R], BF16, tag="qs", name="qs")
                nc.scalar.activation(
                    qs[:], ML[:], mybir.ActivationFunctionType.Sign,
                    bias=th2_ap(t), scale=-1.0,
                )
                qL = qpool.tile([128, NG * R], BF16, tag="qL", name="qL")
                nc.scalar.activation(
                    qL[:], qs[:], mybir.ActivationFunctionType.Relu,
                    bias=zero_ap, scale=1.0,
                )
                spg = qpool.tile([128, NG * R], BF16, tag="sp2", name="spg")
                nc.scalar.activation(
                    spg[:], qs[:], mybir.ActivationFunctionType.Relu,
                    bias=zero_ap, scale=-1.0,
                )
                nc.vector.tensor_tensor(ML[:], ML[:], qL[:], OP.mult)

                # --- layer 2: out += sp2 @ W2.T (persistent psum) ---
                for k in range(NG):
                    nc.tensor.matmul(
                        o_psum[:],
                        lhsT=w2T(k),
                        rhs=spg[:, k * R : (k + 1) * R],
                        start=(t == 0 and k == 0),
                        stop=(t == T - 1 and k == NG - 1),
                    )

            # ---- prologue: pair 0 load + apical ----
            te_tiles, last_te_dma = emit_te_dma(0, chain=True)
            mega = appool.tile([128, NG * 512], F32, tag="ap", name="ap_mega")
            for c in range(NCHUNK):
                emit_ap_chunk(mega, te_tiles, c)
            dM = nc.sync.dma_start(wallM_sb[:], wallM[:])
            add_dep_helper(dM.ins, last_te_dma.ins, reason="wallM after te0 chain")
            dB = nc.sync.dma_start(wallB_sb[:], wallB[:])
            add_dep_helper(dB.ins, dM.ins, reason="wallB after wallM")
            dC = nc.sync.dma_start(cons_sb[:], cons[:])
            add_dep_helper(dC.ins, dM.ins, reason="cons after wallM")
            emit_basal()

            # ---- software-pipelined main loop ----
            for pair in range(NPAIR):
                emit_a_updates(mega, pair)
                if pair + 1 < NPAIR:
                    te_tiles, _ = emit_te_dma(pair + 1, chain=False)
                    meganxt = appool.tile([128, NG * 512], F32, tag="ap",
                                          name="ap_mega")
                    emit_ap_chunk(meganxt, te_tiles, 0)
                    emit_ap_chunk(meganxt, te_tiles, 1)
                    q0 = emit_sub_m(pair, 0)
                    emit_ap_chunk(meganxt, te_tiles, 2)
                    emit_sub_h(pair, 0, q0)
                    emit_ap_chunk(meganxt, te_tiles, 3)
                    q1 = emit_sub_m(pair, 1)
                    emit_ap_chunk(meganxt, te_tiles, 4)
                    emit_ap_chunk(meganxt, te_tiles, 5)
                    emit_sub_h(pair, 1, q1)
                    mega = meganxt
                else:
                    q0 = emit_sub_m(pair, 0)
                    emit_sub_h(pair, 0, q0)
                    q1 = emit_sub_m(pair, 1)
                    emit_sub_h(pair, 1, q1)

            # ---- final eviction: out = o_psum / T + b2 ----
            out_sb = state.tile([L, R], F32, tag="out_sb", name="out_sb")
            nc.scalar.activation(
                out_sb[:], o_psum[:],
                mybir.ActivationFunctionType.Identity,
                bias=ev_bias, scale=1.0 / T,
            )
            nc.sync.dma_start(out[:], out_sb[:])

    return nc


def _f8(a):
    """fp32 -> TRN fp8e4 (IEEE e4m3, max 240) with clip."""
    return np.clip(a, -240.0, 240.0).astype(ml_dtypes.float8_e4m3)


def prep_in_maps(inputs):
    """Host-side shard + transpose + pad + cast. Returns list of per-core dicts."""
    se = np.asarray(inputs["state_embedding"], np.float32)
    te = np.asarray(inputs["tau_embedding"], np.float32)
    Wb = np.asarray(inputs["Wb"], np.float32)
    Wa = np.asarray(inputs["Wa"], np.float32)
    W1 = np.asarray(inputs["W1"], np.float32)
    b1 = np.asarray(inputs["b1"], np.float32)
    W2 = np.asarray(inputs["W2"], np.float32)
    b2 = np.asarray(inputs["b2"], np.float32)
    bf = ml_dtypes.bfloat16

    def padk(a, kd):  # pad feature axis 0
        o = np.zeros((kd,) + a.shape[1:], a.dtype)
        o[: a.shape[0]] = a
        return o

    def swz(a, cols):  # [KDB, cols] -> [128, NKB*cols] bf16 k-major
        nk = a.shape[0] // 128
        return np.ascontiguousarray(
            a.reshape(nk, 128, cols).transpose(1, 0, 2).reshape(128, nk * cols)
            .astype(bf)
        )

    # wallA (fp8): col = k256*1024 + g*256 + kk*128 + m
    WaT64 = padk(np.ascontiguousarray(Wa.T) * WSCALE, KD2)       # [KD2, F]
    wallA = np.ascontiguousarray(
        WaT64.reshape(NK2, 2, 128, NG, 128)      # [k, kk, p, g, m]
        .transpose(2, 0, 3, 1, 4)                # [p, k, g, kk, m]
        .reshape(128, NK2 * 1024)
    )
    wallA = _f8(wallA)

    wallM_wb = swz(padk(np.ascontiguousarray(Wb.T), KDB), F)
    wallB = np.zeros((128, WB_COLS), bf)
    wallB[:, O_W1 : O_W1 + NG * H] = swz(np.ascontiguousarray(W1.T), H)
    wallB[:, O_W2 : O_W2 + NG * L] = swz(np.ascontiguousarray(W2.T), L)
    c1 = (W1.sum(axis=1) + b1).astype(np.float32)
    wallB[0, O_AUG : O_AUG + NG * 128] = (-c1).astype(bf)

    cons = np.zeros((128, 2 + T), np.float32)
    cons[:L, 0] = b2
    for t in range(T):
        cons[:, 2 + t] = 2.0 ** t

    in_maps = []
    for i in range(N_CORES):
        # teT fp8: [pair, p, k128*512 + sub*R + r] ; k128 = 2*k256+kk
        tei = te[:, i * R : (i + 1) * R, :]       # [T, R, DT]
        tei = tei.reshape(NPAIR, 2 * R, DT)
        tei_p = np.zeros((NPAIR, 2 * R, KD2), np.float32)
        tei_p[:, :, :DT] = tei
        teT = np.ascontiguousarray(
            tei_p.reshape(NPAIR, 2 * R, 2 * NK2, 128)
            .transpose(0, 3, 2, 1)                # [pair, p, k128, n]
            .reshape(NPAIR, 128, NK2 * 1024)
        )
        teT = _f8(teT)
        # seT region of wallM: [p, k*T*NB + t*NB+b] = se[t, batch, d]
        sei = se[:, i * NB : (i + 1) * NB, :]     # [T, NB, DS]
        seTt = padk(np.ascontiguousarray(sei.reshape(T * NB, DS).T), KDB)
        wallM_i = np.empty((128, WM_COLS), bf)
        wallM_i[:, O_WB : O_WB + NKB * F] = wallM_wb
        wallM_i[:, O_SE : O_SE + NKB * T * NB] = swz(seTt, T * NB)
        in_maps.append(dict(teT=teT, wallA=wallA, wallM=wallM_i,
                            wallB=wallB, cons=cons))
    return in_maps


def assemble_out(core_outs):
    """[N_CORES][L, R] -> [B, L, S]"""
    full = np.stack([np.asarray(o, np.float32) for o in core_outs], axis=0)
    full = full.reshape(N_CORES, L, NB, S).transpose(0, 2, 1, 3)
    return np.ascontiguousarray(full.reshape(B, L, S))


_NC_CACHE = {}


def get_nc():
    key = "nc"
    if key not in _NC_CACHE:
        last = None
        for _ in range(6):
            try:
                _NC_CACHE[key] = build_nc()
                break
            except Exception as e:  # rare scheduler-order race-detector trip
                last = e
        else:
            raise last
    return _NC_CACHE[key]


def run_sharded(in_maps, trace=False, **kw):
    nc = get_nc()
    if not getattr(nc, "_waits_split", False):
        _split_excess_waits(nc)
        nc._waits_split = True
    res = run_bass_kernel_spmd(
        nc, in_maps, core_ids=list(range(N_CORES)), trace=trace, **kw
    )
    return res


def kernel(**inputs):
    in_maps = prep_in_maps(inputs)
    res = run_sharded(in_maps)
    return assemble_out([res.results[i]["out"] for i in range(N_CORES)])
